# revision 16
# baseline (speedup 1.0000x reference)
"""Causal single-head attention (B=8, S=2048, D=2048, H=128) on 8 TRN2 NeuronCores.

Strategy: data-parallel over batch — core b computes batch element b entirely
on-chip; no collectives. Host-side staging does the layout work the PE used to
do on-device:

  - x is pre-transposed and pre-cast on the host: xt[sr, p, dc, q] =
    x[sr*512+q, dc*128+p] in bf16, so the QKV contraction operand streams
    straight from DRAM in [d-part, s] layout (no on-device transposes, half
    the HBM bytes of the f32 original). DMAs use 4-8KB per-partition lines
    (few instructions, near-peak HBM bandwidth), split across the otherwise
    idle Vector and Sync queues.
  - weights pre-packed [p, dc, h] bf16, loaded via the Scalar queue.

Per core:
  - Q^T, K^T, V^T [h, s] via matmuls with Wq/Wk/Wv chunks stationary; V
    rechunked to [k, h] by PE transposes, with a ones-column appended
    (v1 = [V | 1]).
  - scores^T [k, q] = (lhsT=K^T chunk).T @ Q^T slice; exp on ScalarE with the
    1/sqrt(H) scale folded in. Causal handling: upper-triangle chunks skipped,
    diagonal chunks computed only on their live column range (fully-masked
    columns never touch the PE or ScalarE) with the 128-wide partial triangle
    zeroed post-exp (gpsimd affine_select).
  - AV computed in [q, h] layout: lhsT = exp-tile slice [k, q-sub] stationary,
    rhs = v1 [k, 129] moving -> av[q, 0:128] is the unnormalized output and
    av[q, 128] is the softmax denominator, accumulated by the same matmuls.
    AV trails scores/exp by AV_LAG chunks so exp latency never stalls the PE.
  - ScalarE exp (~720ns/chunk) outpaces the PE's ~450ns/chunk of attention
    work, so the next s-range's QKV matmuls are interleaved INTO each
    attention block as PE filler; block 3 (which has no successor) runs
    diagonal-last so its own K/V projection is the filler.
  - Epilogue per q-block: one DVE reciprocal of the denominator column, a
    per-partition broadcast mul on ScalarE, DMA out. No transposes, no
    reduction chains.

All matmuls bf16 (f32 PSUM accumulation); rel err vs the f32 reference ~5e-3.
"""

import numpy as np
import ml_dtypes

import concourse.bass as bass
import concourse.mybir as mybir
import concourse.tile as tile
from concourse import bacc
from concourse.bass_utils import run_bass_kernel_spmd
from concourse.masks import make_identity

B, S, D, H = 8, 2048, 2048, 128
P = 128
DC = D // P            # 16 d-chunks (contraction)
SC = S // P            # 16 s-chunks
QB = 512               # q-block (moving free dim)
NQ = S // QB           # 4 q-blocks
SPB = QB // P          # 4 s-chunks per q-block
H1 = H + 1             # V with ones column appended (denominator trick)
SCALE = float(H) ** -0.5
AV_LAG = 6             # AV trails scores/exp by this many k-chunks
WARMUP = 16            # dummy matmuls to ramp the HAM clock gate

F32 = mybir.dt.float32
BF16 = mybir.dt.bfloat16

_NC_CACHE = None


def build():
    nc = bacc.Bacc(None, target_bir_lowering=False)

    xt_d = nc.declare_dram_parameter("xt", [NQ, P, DC, QB], BF16, isOutput=False)
    wq_d = nc.declare_dram_parameter("wq", [P, DC, H], BF16, isOutput=False)
    wk_d = nc.declare_dram_parameter("wk", [P, DC, H], BF16, isOutput=False)
    wv_d = nc.declare_dram_parameter("wv", [P, DC, H], BF16, isOutput=False)
    out_d = nc.declare_dram_parameter("out", [S, H], F32, isOutput=True)

    with tile.TileContext(nc) as tc:
        with (
            tc.tile_pool(name="const", bufs=1) as const,
            tc.tile_pool(name="persist", bufs=1) as persist,
            tc.tile_pool(name="qv", bufs=2) as qv_pool,
            tc.tile_pool(name="et", bufs=20) as et_pool,
            tc.tile_pool(name="epi", bufs=4) as epi_pool,
            tc.tile_pool(name="ps_qkv", bufs=2, space="PSUM") as ps_qkv,
            tc.tile_pool(name="ps_sc", bufs=2, space="PSUM") as ps_sc,
            tc.tile_pool(name="ps_av", bufs=1, space="PSUM") as ps_av,
        ):
            # warmup operand first on the Vector queue so the dummy matmuls
            # depend on nothing else -- PE busy right after the preamble
            junk = const.tile([P, P], BF16, tag="junk")
            nc.vector.memset(junk[:], 0.5)

            # weights on the Scalar queue (idle until the first exp), x split
            # across Vector and Sync queues -- three parallel DMA streams.
            # Load order matches first use: wq / wv / wk.
            w_sb = {}
            for name, wd in (("wq", wq_d), ("wv", wv_d), ("wk", wk_d)):
                t = const.tile([P, DC, H], BF16, tag=f"w_{name}", name=f"w_{name}")
                nc.scalar.dma_start(out=t[:], in_=wd.ap())
                w_sb[name] = t

            xt = [persist.tile([P, DC, QB], BF16, tag=f"xt{sr}", name=f"xt{sr}")
                  for sr in range(NQ)]
            # sr0 in 4KB-line quarters for early QKV start; the rest in
            # 8KB-line halves (fewer issue slots, near-peak bandwidth)
            for g in range(4):
                nc.sync.dma_start(
                    out=xt[0][:, g * 4 : (g + 1) * 4, :],
                    in_=xt_d.ap()[0, :, g * 4 : (g + 1) * 4, :],
                )
            for sr, eng in ((1, nc.sync), (2, nc.gpsimd), (3, nc.gpsimd)):
                for g in range(2):
                    eng.dma_start(
                        out=xt[sr][:, g * 8 : (g + 1) * 8, :],
                        in_=xt_d.ap()[sr, :, g * 8 : (g + 1) * 8, :],
                    )

            for _ in range(WARMUP):
                wu_ps = ps_sc.tile([P, P], F32, tag="sc", name="wu_ps")
                nc.tensor.matmul(wu_ps[:], junk[:], junk[:], start=True, stop=True)

            ident_bf = const.tile([P, P], BF16, tag="ident_bf")
            make_identity(nc, ident_bf[:])

            q_sb = [persist.tile([P, QB], BF16, tag=f"q_sb{i}", name=f"q_sb{i}")
                    for i in range(NQ)]
            k_sb = [persist.tile([P, QB], BF16, tag=f"k_sb{i}", name=f"k_sb{i}")
                    for i in range(NQ)]
            # v1 = [V | 1]: cols 0..127 are V chunks [k, h], col 128 is ones
            # so the AV matmuls accumulate softmax denominators for free
            v1_sb = persist.tile([P, SC, H1], BF16, tag="v1_sb")
            nc.gpsimd.memset(v1_sb[:, :, H:H1], 1.0)

            vt_tiles = {}

            def qkv_units(sr, parts):
                """List of emission thunks for s-range sr's projections.

                parts is a subset of 'q', 'v', 'k', 'r' (r = V rechunk).
                Emitted inline or interleaved into an attention block as PE
                filler.
                """
                units = []

                def proj(w_t, dst):
                    pr_ps_box = [None]

                    def mk(dc):
                        def f():
                            if dc == 0:
                                pr_ps_box[0] = ps_qkv.tile(
                                    [P, QB], F32, tag="qkv", name="pr_ps"
                                )
                            nc.tensor.matmul(
                                pr_ps_box[0][:], w_t[:, dc, :], xt[sr][:, dc, :],
                                start=(dc == 0), stop=(dc == DC - 1),
                            )
                        return f

                    for dc in range(DC):
                        units.append(mk(dc))
                    units.append(lambda: nc.vector.tensor_copy(dst[:], pr_ps_box[0][:]))

                if "q" in parts:
                    proj(w_sb["wq"], q_sb[sr])
                if "v" in parts:
                    vt = qv_pool.tile([P, QB], BF16, tag="vt", name="vt")
                    vt_tiles[sr] = vt
                    proj(w_sb["wv"], vt)
                if "k" in parts:
                    proj(w_sb["wk"], k_sb[sr])
                if "r" in parts:
                    tp_box = [None]

                    def mk_tr(sj):
                        def f():
                            if sj == 0:
                                tp_box[0] = ps_sc.tile(
                                    [P, SPB, P], BF16, tag="sc", name="tp_v"
                                )
                            nc.tensor.transpose(
                                tp_box[0][:, sj, :],
                                vt_tiles[sr][:, sj * P : (sj + 1) * P],
                                ident_bf[:],
                            )
                        return f

                    for sj in range(SPB):
                        units.append(mk_tr(sj))
                    units.append(lambda: nc.vector.tensor_copy(
                        v1_sb[:, sr * SPB : (sr + 1) * SPB, 0:H], tp_box[0][:]
                    ))
                return units

            def attention(qb, filler, diag_last=False):
                nkc = SPB * (qb + 1)
                olds = list(range(qb * SPB))
                diags = list(range(qb * SPB, nkc))
                if qb == 0:
                    order = diags
                elif diag_last:
                    # block 3: its own K/V projection is the filler, so the
                    # diagonal chunks (which need fresh K) go last
                    order = olds + diags
                else:
                    # a couple of old-K chunks first so the first scores
                    # matmul isn't gated on this range's K copy
                    order = olds[:2] + diags + olds[2:]

                # one PSUM bank per q-sub: accumulation regions held open
                # across the block must not share a bank
                av = [
                    ps_av.tile([P, H1], F32, tag=f"av{i}", name=f"av{i}")
                    for i in range(SPB)
                ]
                e_tiles = [None] * nkc

                def qs_list(kc):
                    # diagonal chunk kc: columns below (kc%SPB)*P are fully
                    # masked -- skip those q-subs entirely
                    return range((kc - qb * SPB) if kc >= qb * SPB else 0, SPB)

                first_pos = {}
                last_pos = {}
                for pos, kc in enumerate(order):
                    for qs in qs_list(kc):
                        if qs not in first_pos:
                            first_pos[qs] = pos
                        last_pos[qs] = pos

                def emit_av(pos):
                    kc = order[pos]
                    for qs in qs_list(kc):
                        nc.tensor.matmul(
                            av[qs][:],
                            e_tiles[kc][:, qs * P : (qs + 1) * P],
                            v1_sb[:, kc, :],
                            start=(pos == first_pos[qs]),
                            stop=(pos == last_pos[qs]),
                        )

                fl = list(filler)
                fi = 0
                for pos, kc in enumerate(order):
                    diag = kc >= qb * SPB
                    c0 = (kc - qb * SPB) * P if diag else 0
                    sc_ps = ps_sc.tile([P, QB], F32, tag="sc", name="sc_ps")
                    nc.tensor.matmul(
                        sc_ps[:, c0:QB],
                        k_sb[kc // SPB][:, (kc % SPB) * P : (kc % SPB + 1) * P],
                        q_sb[qb][:, c0:QB],
                        start=True,
                        stop=True,
                    )
                    e_t = et_pool.tile([P, QB], BF16, tag="et", name="e_t")
                    e_tiles[kc] = e_t
                    nc.scalar.activation(
                        e_t[:, c0:QB], sc_ps[:, c0:QB],
                        mybir.ActivationFunctionType.Exp, scale=SCALE,
                    )
                    if diag:
                        # partial triangle lives only in cols [c0, c0+P):
                        # keep j' >= p within that slab, zero otherwise
                        nc.gpsimd.affine_select(
                            out=e_t[:, c0 : c0 + P],
                            in_=e_t[:, c0 : c0 + P],
                            compare_op=mybir.AluOpType.is_ge,
                            fill=0.0,
                            base=0,
                            pattern=[[1, P]],
                            channel_multiplier=-1,
                        )
                    # interleave next-range QKV matmuls as PE filler so the
                    # exp pipeline (the attention bottleneck) never stalls PE
                    want = (len(fl) - fi + (nkc - 1 - pos)) // max(1, nkc - pos)
                    for _ in range(want):
                        fl[fi]()
                        fi += 1
                    if pos >= AV_LAG:
                        emit_av(pos - AV_LAG)
                while fi < len(fl):
                    fl[fi]()
                    fi += 1
                for pos in range(max(0, nkc - AV_LAG), nkc):
                    emit_av(pos)

                # epilogue: denominators live in av[qs][:, 128]; reciprocal
                # on DVE, broadcast-mul on ScalarE, DMA out via Sync
                rs = epi_pool.tile([P, SPB], F32, tag="rs", name="rs")
                for qs in range(SPB):
                    nc.vector.reciprocal(rs[:, qs : qs + 1], av[qs][:, H:H1])
                for qs in range(SPB):
                    out_sb = epi_pool.tile([P, H], F32, tag="out_sb", name="out_sb")
                    nc.scalar.mul(out_sb[:], av[qs][:, 0:H], rs[:, qs : qs + 1])
                    nc.sync.dma_start(
                        out=out_d[(qb * QB + qs * P) : (qb * QB + (qs + 1) * P), :],
                        in_=out_sb[:],
                    )

            # ---- main pipeline ----
            for u in qkv_units(0, "qvkr"):
                u()
            attention(0, qkv_units(1, "q"))
            for u in qkv_units(1, "vkr"):
                u()
            attention(1, qkv_units(2, "qvkr"))
            attention(2, qkv_units(3, "qv"))
            attention(3, qkv_units(3, "kr"), diag_last=True)

    nc.compile()
    return nc


def make_in_maps(x, Wq, Wk, Wv):
    x = np.ascontiguousarray(x, dtype=np.float32)

    def pack_w(w):
        w = np.asarray(w, dtype=np.float32).astype(ml_dtypes.bfloat16)
        return np.ascontiguousarray(w.reshape(DC, P, H).transpose(1, 0, 2))

    wq_p, wk_p, wv_p = pack_w(Wq), pack_w(Wk), pack_w(Wv)
    in_maps = []
    for b in range(B):
        xt = (
            x[b]
            .astype(ml_dtypes.bfloat16)
            .reshape(NQ, QB, DC, P)
            .transpose(0, 3, 2, 1)
        )
        in_maps.append(
            {
                "xt": np.ascontiguousarray(xt),
                "wq": wq_p,
                "wk": wk_p,
                "wv": wv_p,
            }
        )
    return in_maps


def gather_out(res):
    return np.stack([res.results[b]["out"] for b in range(B)]).astype(np.float32)


def kernel(x, Wq, Wk, Wv):
    global _NC_CACHE
    if _NC_CACHE is None:
        _NC_CACHE = build()
    nc = _NC_CACHE
    res = run_bass_kernel_spmd(nc, make_in_maps(x, Wq, Wk, Wv), core_ids=list(range(B)))
    return gather_out(res)


# revision 17
# speedup vs baseline: 1.1314x; 1.1314x over previous
"""Causal single-head attention (B=8, S=2048, D=2048, H=128) on 8 TRN2 NeuronCores.

Strategy: data-parallel over batch — core b computes batch element b entirely
on-chip; no collectives. Host-side staging does the layout work the PE used to
do on-device:

  - x is pre-transposed and pre-cast on the host: xt[sr, p, dc, q] =
    x[sr*512+q, dc*128+p] in bf16, so the QKV contraction operand streams
    straight from DRAM in [d-part, s] layout (no on-device transposes, half
    the HBM bytes of the f32 original). DMAs use 4-8KB per-partition lines
    (few instructions, near-peak HBM bandwidth), split across the otherwise
    idle Vector and Sync queues.
  - weights pre-packed [p, dc, h] bf16, loaded via the Scalar queue.

Per core:
  - Q^T, K^T, V^T [h, s] via matmuls with Wq/Wk/Wv chunks stationary; V
    rechunked to [k, h] by PE transposes, with a ones-column appended
    (v1 = [V | 1]).
  - scores^T [k, q] = (lhsT=K^T chunk).T @ Q^T slice; exp on ScalarE with the
    1/sqrt(H) scale folded in. Causal handling: upper-triangle chunks skipped,
    diagonal chunks computed only on their live column range (fully-masked
    columns never touch the PE or ScalarE) with the 128-wide partial triangle
    zeroed post-exp (gpsimd affine_select).
  - AV computed in [q, h] layout: lhsT = exp-tile slice [k, q-sub] stationary,
    rhs = v1 [k, 129] moving -> av[q, 0:128] is the unnormalized output and
    av[q, 128] is the softmax denominator, accumulated by the same matmuls.
    AV trails scores/exp by AV_LAG chunks so exp latency never stalls the PE.
  - ScalarE exp (~720ns/chunk) outpaces the PE's ~450ns/chunk of attention
    work, so the next s-range's QKV matmuls are interleaved INTO each
    attention block as PE filler; block 3 (which has no successor) runs
    diagonal-last so its own K/V projection is the filler.
  - Epilogue per q-block: one DVE reciprocal of the denominator column, a
    per-partition broadcast mul on ScalarE, DMA out. No transposes, no
    reduction chains.

All matmuls bf16 (f32 PSUM accumulation); rel err vs the f32 reference ~5e-3.
"""

import numpy as np
import ml_dtypes

import concourse.bass as bass
import concourse.mybir as mybir
import concourse.tile as tile
from concourse import bacc
from concourse.bass_utils import run_bass_kernel_spmd
from concourse.masks import make_identity

B, S, D, H = 8, 2048, 2048, 128
P = 128
DC = D // P            # 16 d-chunks (contraction)
SC = S // P            # 16 s-chunks
QB = 512               # q-block (moving free dim)
NQ = S // QB           # 4 q-blocks
SPB = QB // P          # 4 s-chunks per q-block
H1 = H + 1             # V with ones column appended (denominator trick)
SCALE = float(H) ** -0.5
AV_LAG = 6             # AV trails scores/exp by this many k-chunks
WARMUP = 16            # dummy matmuls to ramp the HAM clock gate

F32 = mybir.dt.float32
BF16 = mybir.dt.bfloat16

_NC_CACHE = None


def build():
    nc = bacc.Bacc(None, target_bir_lowering=False)

    xt_d = nc.declare_dram_parameter("xt", [NQ, P, DC, QB], BF16, isOutput=False)
    wq_d = nc.declare_dram_parameter("wq", [P, DC, H], BF16, isOutput=False)
    wk_d = nc.declare_dram_parameter("wk", [P, DC, H], BF16, isOutput=False)
    wv_d = nc.declare_dram_parameter("wv", [P, DC, H], BF16, isOutput=False)
    out_d = nc.declare_dram_parameter("out", [S, H], F32, isOutput=True)

    with tile.TileContext(nc) as tc:
        with (
            tc.tile_pool(name="const", bufs=1) as const,
            tc.tile_pool(name="persist", bufs=1) as persist,
            tc.tile_pool(name="qv", bufs=2) as qv_pool,
            tc.tile_pool(name="et", bufs=20) as et_pool,
            tc.tile_pool(name="epi", bufs=4) as epi_pool,
            tc.tile_pool(name="ps_qkv", bufs=2, space="PSUM") as ps_qkv,
            tc.tile_pool(name="ps_sc", bufs=2, space="PSUM") as ps_sc,
            tc.tile_pool(name="ps_av", bufs=1, space="PSUM") as ps_av,
        ):
            # warmup operand first on the Vector queue so the dummy matmuls
            # depend on nothing else -- PE busy right after the preamble
            junk = const.tile([P, P], BF16, tag="junk")
            nc.vector.memset(junk[:], 0.5)

            # All input DMAs go through the GpSimd SWDGE queue -- measured
            # ~2.5x the sustained bandwidth of the Sync/Scalar HWDGE queues.
            # Load order matches first use: wq / wv / wk.
            w_sb = {}
            for name, wd in (("wq", wq_d), ("wv", wv_d), ("wk", wk_d)):
                t = const.tile([P, DC, H], BF16, tag=f"w_{name}", name=f"w_{name}")
                nc.gpsimd.dma_start(out=t[:], in_=wd.ap())
                w_sb[name] = t

            # constants next on the gpsimd queue, before the bulk x stream,
            # so the first rechunk/mask aren't stuck behind 8us of DMA issue
            ident_bf = const.tile([P, P], BF16, tag="ident_bf")
            make_identity(nc, ident_bf[:])
            # v1 = [V | 1]: cols 0..127 are V chunks [k, h], col 128 is ones
            # so the AV matmuls accumulate softmax denominators for free
            v1_sb = persist.tile([P, SC, H1], BF16, tag="v1_sb")
            nc.gpsimd.memset(v1_sb[:, :, H:H1], 1.0)

            xt = [persist.tile([P, DC, QB], BF16, tag=f"xt{sr}", name=f"xt{sr}")
                  for sr in range(NQ)]
            # sr0 in 4KB-line quarters for early QKV start; the rest in
            # 8KB-line halves (fewer issue slots, near-peak bandwidth)
            for g in range(4):
                nc.gpsimd.dma_start(
                    out=xt[0][:, g * 4 : (g + 1) * 4, :],
                    in_=xt_d.ap()[0, :, g * 4 : (g + 1) * 4, :],
                )
            for sr in range(1, NQ):
                for g in range(2):
                    nc.gpsimd.dma_start(
                        out=xt[sr][:, g * 8 : (g + 1) * 8, :],
                        in_=xt_d.ap()[sr, :, g * 8 : (g + 1) * 8, :],
                    )

            for _ in range(WARMUP):
                wu_ps = ps_sc.tile([P, P], F32, tag="sc", name="wu_ps")
                nc.tensor.matmul(wu_ps[:], junk[:], junk[:], start=True, stop=True)

            q_sb = [persist.tile([P, QB], BF16, tag=f"q_sb{i}", name=f"q_sb{i}")
                    for i in range(NQ)]
            k_sb = [persist.tile([P, QB], BF16, tag=f"k_sb{i}", name=f"k_sb{i}")
                    for i in range(NQ)]

            vt_tiles = {}

            def qkv_units(sr, parts):
                """List of emission thunks for s-range sr's projections.

                parts is a subset of 'q', 'v', 'k', 'r' (r = V rechunk).
                Emitted inline or interleaved into an attention block as PE
                filler.
                """
                units = []

                def proj(w_t, dst):
                    pr_ps_box = [None]

                    def mk(dc):
                        def f():
                            if dc == 0:
                                pr_ps_box[0] = ps_qkv.tile(
                                    [P, QB], F32, tag="qkv", name="pr_ps"
                                )
                            nc.tensor.matmul(
                                pr_ps_box[0][:], w_t[:, dc, :], xt[sr][:, dc, :],
                                start=(dc == 0), stop=(dc == DC - 1),
                            )
                        return f

                    for dc in range(DC):
                        units.append(mk(dc))
                    units.append(lambda: nc.vector.tensor_copy(dst[:], pr_ps_box[0][:]))

                if "q" in parts:
                    proj(w_sb["wq"], q_sb[sr])
                if "v" in parts:
                    vt = qv_pool.tile([P, QB], BF16, tag="vt", name="vt")
                    vt_tiles[sr] = vt
                    proj(w_sb["wv"], vt)
                if "k" in parts:
                    proj(w_sb["wk"], k_sb[sr])
                if "r" in parts:
                    tp_box = [None]

                    def mk_tr(sj):
                        def f():
                            if sj == 0:
                                tp_box[0] = ps_sc.tile(
                                    [P, SPB, P], BF16, tag="sc", name="tp_v"
                                )
                            nc.tensor.transpose(
                                tp_box[0][:, sj, :],
                                vt_tiles[sr][:, sj * P : (sj + 1) * P],
                                ident_bf[:],
                            )
                        return f

                    for sj in range(SPB):
                        units.append(mk_tr(sj))
                    units.append(lambda: nc.vector.tensor_copy(
                        v1_sb[:, sr * SPB : (sr + 1) * SPB, 0:H], tp_box[0][:]
                    ))
                return units

            def attention(qb, filler, diag_last=False):
                nkc = SPB * (qb + 1)
                olds = list(range(qb * SPB))
                diags = list(range(qb * SPB, nkc))
                if qb == 0:
                    order = diags
                elif diag_last:
                    # block 3: its own K/V projection is the filler, so the
                    # diagonal chunks (which need fresh K) go last
                    order = olds + diags
                else:
                    # a couple of old-K chunks first so the first scores
                    # matmul isn't gated on this range's K copy
                    order = olds[:2] + diags + olds[2:]

                # one PSUM bank per q-sub: accumulation regions held open
                # across the block must not share a bank
                av = [
                    ps_av.tile([P, H1], F32, tag=f"av{i}", name=f"av{i}")
                    for i in range(SPB)
                ]
                e_tiles = [None] * nkc

                def qs_list(kc):
                    # diagonal chunk kc: columns below (kc%SPB)*P are fully
                    # masked -- skip those q-subs entirely
                    return range((kc - qb * SPB) if kc >= qb * SPB else 0, SPB)

                first_pos = {}
                last_pos = {}
                for pos, kc in enumerate(order):
                    for qs in qs_list(kc):
                        if qs not in first_pos:
                            first_pos[qs] = pos
                        last_pos[qs] = pos

                def emit_av(pos):
                    kc = order[pos]
                    for qs in qs_list(kc):
                        nc.tensor.matmul(
                            av[qs][:],
                            e_tiles[kc][:, qs * P : (qs + 1) * P],
                            v1_sb[:, kc, :],
                            start=(pos == first_pos[qs]),
                            stop=(pos == last_pos[qs]),
                        )

                fl = list(filler)
                fi = 0
                for pos, kc in enumerate(order):
                    diag = kc >= qb * SPB
                    c0 = (kc - qb * SPB) * P if diag else 0
                    sc_ps = ps_sc.tile([P, QB], F32, tag="sc", name="sc_ps")
                    nc.tensor.matmul(
                        sc_ps[:, c0:QB],
                        k_sb[kc // SPB][:, (kc % SPB) * P : (kc % SPB + 1) * P],
                        q_sb[qb][:, c0:QB],
                        start=True,
                        stop=True,
                    )
                    e_t = et_pool.tile([P, QB], BF16, tag="et", name="e_t")
                    e_tiles[kc] = e_t
                    nc.scalar.activation(
                        e_t[:, c0:QB], sc_ps[:, c0:QB],
                        mybir.ActivationFunctionType.Exp, scale=SCALE,
                    )
                    if diag:
                        # partial triangle lives only in cols [c0, c0+P):
                        # keep j' >= p within that slab, zero otherwise
                        nc.gpsimd.affine_select(
                            out=e_t[:, c0 : c0 + P],
                            in_=e_t[:, c0 : c0 + P],
                            compare_op=mybir.AluOpType.is_ge,
                            fill=0.0,
                            base=0,
                            pattern=[[1, P]],
                            channel_multiplier=-1,
                        )
                    # interleave next-range QKV matmuls as PE filler so the
                    # exp pipeline (the attention bottleneck) never stalls PE
                    want = (len(fl) - fi + (nkc - 1 - pos)) // max(1, nkc - pos)
                    for _ in range(want):
                        fl[fi]()
                        fi += 1
                    if pos >= AV_LAG:
                        emit_av(pos - AV_LAG)
                while fi < len(fl):
                    fl[fi]()
                    fi += 1
                for pos in range(max(0, nkc - AV_LAG), nkc):
                    emit_av(pos)

                # epilogue: denominators live in av[qs][:, 128]; reciprocal
                # on DVE, broadcast-mul on ScalarE, DMA out via Sync
                rs = epi_pool.tile([P, SPB], F32, tag="rs", name="rs")
                for qs in range(SPB):
                    nc.vector.reciprocal(rs[:, qs : qs + 1], av[qs][:, H:H1])
                for qs in range(SPB):
                    out_sb = epi_pool.tile([P, H], F32, tag="out_sb", name="out_sb")
                    nc.scalar.mul(out_sb[:], av[qs][:, 0:H], rs[:, qs : qs + 1])
                    nc.sync.dma_start(
                        out=out_d[(qb * QB + qs * P) : (qb * QB + (qs + 1) * P), :],
                        in_=out_sb[:],
                    )

            # ---- main pipeline ----
            for u in qkv_units(0, "qvkr"):
                u()
            attention(0, qkv_units(1, "q"))
            for u in qkv_units(1, "vkr"):
                u()
            attention(1, qkv_units(2, "qvkr"))
            attention(2, qkv_units(3, "qv"))
            attention(3, qkv_units(3, "kr"), diag_last=True)

    nc.compile()
    return nc


def make_in_maps(x, Wq, Wk, Wv):
    x = np.ascontiguousarray(x, dtype=np.float32)

    def pack_w(w):
        w = np.asarray(w, dtype=np.float32).astype(ml_dtypes.bfloat16)
        return np.ascontiguousarray(w.reshape(DC, P, H).transpose(1, 0, 2))

    wq_p, wk_p, wv_p = pack_w(Wq), pack_w(Wk), pack_w(Wv)
    in_maps = []
    for b in range(B):
        xt = (
            x[b]
            .astype(ml_dtypes.bfloat16)
            .reshape(NQ, QB, DC, P)
            .transpose(0, 3, 2, 1)
        )
        in_maps.append(
            {
                "xt": np.ascontiguousarray(xt),
                "wq": wq_p,
                "wk": wk_p,
                "wv": wv_p,
            }
        )
    return in_maps


def gather_out(res):
    return np.stack([res.results[b]["out"] for b in range(B)]).astype(np.float32)


def kernel(x, Wq, Wk, Wv):
    global _NC_CACHE
    if _NC_CACHE is None:
        _NC_CACHE = build()
    nc = _NC_CACHE
    res = run_bass_kernel_spmd(nc, make_in_maps(x, Wq, Wk, Wv), core_ids=list(range(B)))
    return gather_out(res)
